# revision 13
# baseline (speedup 1.0000x reference)
"""L2 contrastive loss (margin=1.0) on 8 Trainium2 NeuronCores.

loss = (sum_{i!=j} relu(1 - d_ij)^2 + sum_i d_ii^2) / (2N),
d_ij = ||f1_i - f2_j||.

Sharding: row-shard feature1 across the 8 cores; every core sees all of
feature2 and computes its 1024 x 8192 block of the distance matrix.

Device algorithm per core:
  * PE (bf16): psum = 2 * f1_i . f2_j for a [128 x 2048] supertile.
  * Screen: every element is passed through
        relu(psum + (1 - sq1_i - min_tile sq2_j))
    with the per-partition bias column precomputed on host (feature2 is
    sorted by sq2 so the per-tile min is tight).  Since
    psum + bias >= 2dot + 1 - sq1_i - sq2_j = 1 - d2_ij, the accumulated
    screen is a CONSERVATIVE certificate: screen == 0  ==>  every
    d2_ij >= 1  ==>  every hinge term relu(1 - d_ij) is exactly 0.
    The work is split between DVE (tensor_scalar max+accum) and ACT
    (Relu + accum) to use both engines.
  * Diagonal: sum_i ||f1_i - f2_i||^2 computed exactly in fp32
    (DVE subtract + ACT Square with accumulate), reduced to a scalar
    with a ones-matmul.
Host: loss = sum(diag partials) / (2N) when every core's screen is 0;
otherwise (only if some pair sits within/near the margin) falls back to
an exact full computation.
"""

import numpy as np
import ml_dtypes

N = 8192
D = 128
NCORES = 8
R = N // NCORES  # 1024 rows of feature1 per core

TRACE = False       # test harness can set kernel.TRACE = True
TRACE_KWARGS = {}
LAST_RESULT = None  # BassKernelResults of the last run

_BASS_CACHE = {}

# Supertile layout: 8 i-tiles x 4 j-groups of 2048 -> 32 supertiles.
# Each supertile's screen is split column-wise across DVE and ACT so both
# engines work on every supertile (balanced to their elem/cycle rates).
N_SUPER = 32
NJH = 4
JW = N // NJH  # 2048 j-columns per supertile
DVE_COLS = 1024  # bank-aligned split: DVE 2 banks, ACT 2 banks


def _build_bass():
    import concourse.bacc as bacc
    import concourse.mybir as mybir
    import concourse.tile as tile

    fp32 = mybir.dt.float32
    bf16 = mybir.dt.bfloat16
    Alu = mybir.AluOpType
    Act = mybir.ActivationFunctionType

    nc = bacc.Bacc("TRN2", target_bir_lowering=False, debug=False,
                   num_devices=NCORES)

    # ---- DRAM I/O ----
    # (2*f2_sorted).T in bf16 -- main matmul moving operand
    d_f2t2 = nc.dram_tensor("f2t2", [D, N], bf16, kind="ExternalInput")
    # f1_core.T in bf16 -- main matmul stationary operand
    d_f1t = nc.dram_tensor("f1t", [D, R], bf16, kind="ExternalInput")
    # screen bias columns [128, N_SUPER]: col for supertile (ti, jh) holds
    # 1 - sq1[ti*128 + p] - min_{j in group jh} sq2_j
    d_s1c = nc.dram_tensor("s1c", [128, N_SUPER], fp32, kind="ExternalInput")
    # fp32 rows for the exact diagonal: [:, :R] = f1 rows, [:, R:] = f2 rows
    d_f12 = nc.dram_tensor("f12", [128, 2 * R], fp32, kind="ExternalInput")
    # out[0,0] = sum_i ||f1_i - f2_i||^2 ; out[1,0] = screen (0 iff no hinge)
    d_out = nc.dram_tensor("out", [2, 1], fp32, kind="ExternalOutput")

    with tile.TileContext(nc) as tc:
        with (
            tc.tile_pool(name="singles", bufs=1) as singles,
            tc.tile_pool(name="chunks", bufs=1) as chunks,
        ):
            # ---- input DMAs ----
            s_f12 = singles.tile([128, 2 * R], fp32, tag="f12")
            nc.sync.dma_start(s_f12[:, :], d_f12[:, :])

            s_f1t = singles.tile([D, R], bf16, tag="f1t")
            nc.sync.dma_start(s_f1t[:, :], d_f1t[:, :])
            s_s1c = singles.tile([128, N_SUPER], fp32, tag="s1c")
            nc.sync.dma_start(s_s1c[:, :], d_s1c[:, :])

            # f2t2 in 4 chunks (one per j-group) so matmuls start early;
            # the ti-major supertile order touches all 4 in the first 4
            # supertiles while their DMAs land in parallel queues.
            s_cs = []
            for k in range(NJH):
                t = chunks.tile([D, JW], bf16, tag=f"f2t2_{k}")
                nc.sync.dma_start(t[:, :], d_f2t2[:, k * JW : (k + 1) * JW])
                s_cs.append(t)

            def f2t2_slice(jh, js):
                return s_cs[jh][:, js * 512 : (js + 1) * 512]

            # ---- accumulators & trash ----
            acc_diag = singles.tile([128, 1], fp32, tag="acc_diag")
            acc_d = singles.tile([128, N_SUPER], fp32, tag="acc_d")
            acc_a = singles.tile([128, N_SUPER], fp32, tag="acc_a")
            trash_d = singles.tile([128, JW], bf16, tag="trash_d")
            trash_a = singles.tile([128, JW], bf16, tag="trash_a")
            diff = singles.tile([128, R], fp32, tag="diff")
            trash32 = singles.tile([128, R], fp32, tag="trash32")
            m_final = singles.tile([128, 2], fp32, tag="m_final")
            ones_sb = singles.tile([128, 1], fp32, tag="ones_sb")
            red_d = singles.tile([128, 1], fp32, tag="red_d")
            red_a = singles.tile([128, 1], fp32, tag="red_a")
            out_sb = singles.tile([2, 1], fp32, tag="out_sb")

            nc.vector.memset(ones_sb[:, :], 1.0)

            # ---- exact diagonal: sum_i ||f1_i - f2_i||^2 (fp32) ----
            nc.vector.tensor_sub(diff[:, :], s_f12[:, 0:R], s_f12[:, R : 2 * R])
            nc.scalar.activation(
                trash32[:, :],
                diff[:, :],
                Act.Square,
                accum_out=acc_diag[:, 0:1],
            )

            # ---- main loop ----
            order = [(ti, jh) for ti in range(NCORES) for jh in range(NJH)]
            with tc.tile_pool(name="psum_main", bufs=2, space="PSUM") as pp:
                for ti, jh in order:
                    st = ti * NJH + jh
                    isl = slice(ti * 128, (ti + 1) * 128)
                    ps = pp.tile([128, JW], fp32, tag="ps")
                    # main matmuls: psum = 2 * f1_i . f2_j
                    for js in range(JW // 512):
                        nc.tensor.matmul(
                            ps[:, js * 512 : (js + 1) * 512],
                            lhsT=s_f1t[:, isl],
                            rhs=f2t2_slice(jh, js),
                            start=True,
                            stop=True,
                        )
                    # screen: relu(psum + bias_col) accumulated; zero iff
                    # no hinge term in this supertile.  Column-split across
                    # DVE and ACT so both engines screen every supertile.
                    bias_col = s_s1c[:, st : st + 1]
                    nc.vector.tensor_scalar(
                        trash_d[:, 0:DVE_COLS],
                        ps[:, 0:DVE_COLS],
                        bias_col,
                        0.0,
                        Alu.add,
                        Alu.max,
                        accum_out=acc_d[:, st : st + 1],
                    )
                    nc.scalar.activation(
                        trash_a[:, 0 : JW - DVE_COLS],
                        ps[:, DVE_COLS:JW],
                        Act.Relu,
                        bias=bias_col,
                        scale=1.0,
                        accum_out=acc_a[:, st : st + 1],
                    )

            # ---- final reduction ----
            nc.vector.tensor_reduce(
                red_d[:, :], acc_d[:, :], axis=mybir.AxisListType.X, op=Alu.add
            )
            nc.vector.tensor_reduce(
                red_a[:, :], acc_a[:, :], axis=mybir.AxisListType.X, op=Alu.add
            )
            nc.vector.tensor_copy(m_final[:, 0:1], acc_diag[:, 0:1])
            nc.vector.tensor_add(m_final[:, 1:2], red_d[:, :], red_a[:, :])

            with tc.tile_pool(name="psum_fin", bufs=1, space="PSUM") as pf_pool:
                pf = pf_pool.tile([2, 1], fp32, tag="pf")
                nc.tensor.matmul(
                    pf[:, :], lhsT=m_final[:, :], rhs=ones_sb[:, :],
                    start=True, stop=True,
                )
                nc.vector.tensor_copy(out_sb[:, :], pf[:, :])

            nc.sync.dma_start(d_out[:, :], out_sb[:, :])

    nc.compile()
    return nc


def _get_nc():
    if "nc" not in _BASS_CACHE:
        _BASS_CACHE["nc"] = _build_bass()
    return _BASS_CACHE["nc"]


def _full_numpy_fallback(f1, f2):
    """Exact reference computation (only used if the screen certificate
    fails, i.e. some pair has d_ij close to or inside the margin)."""
    f1 = f1.astype(np.float32)
    f2 = f2.astype(np.float32)
    n = f1.shape[0]
    sq1 = np.sum(f1 * f1, axis=1)
    sq2 = np.sum(f2 * f2, axis=1)
    total = np.float64(0.0)
    chunk = 512
    for s in range(0, n, chunk):
        e = min(s + chunk, n)
        d2 = sq1[s:e, None] + sq2[None, :] - 2.0 * (f1[s:e] @ f2.T)
        d = np.sqrt(np.maximum(d2, 0.0))
        c = np.maximum(1.0 - d, 0.0)
        for r in range(s, e):
            c[r - s, r] = 0.0
        total += np.float64(np.sum(c * c))
    total += np.float64(np.sum((f1 - f2) ** 2))
    return np.float32(total / (2.0 * n))


def kernel(feature1, feature2):
    global LAST_RESULT
    from concourse.bass_utils import run_bass_kernel_spmd

    f1 = np.ascontiguousarray(np.asarray(feature1, dtype=np.float32))
    f2 = np.ascontiguousarray(np.asarray(feature2, dtype=np.float32))
    assert f1.shape == (N, D) and f2.shape == (N, D)

    bf16 = ml_dtypes.bfloat16
    sq1 = np.sum(f1.astype(np.float64) * f1, axis=1)
    sq2 = np.sum(f2.astype(np.float64) * f2, axis=1)

    # Sort feature2 rows by sq2 so the per-supertile min-sq2 bias is tight.
    perm = np.argsort(sq2, kind="stable")
    f2s = f2[perm]
    sq2s = sq2[perm]
    sq2min = sq2s.reshape(NJH, JW).min(axis=1)  # per j-group minimum

    f2t2 = np.ascontiguousarray((2.0 * f2s.T).astype(bf16))           # [D, N]

    in_maps = []
    for c in range(NCORES):
        sl = slice(c * R, (c + 1) * R)
        f1c_rows = f1[sl]                                             # [R, D]
        # bias columns: [128, 32], col (ti*NJH + jh)[p] =
        #   1 - sq1[c*R + ti*128 + p] - sq2min[jh]
        s1c = np.empty((128, N_SUPER), np.float32)
        for ti in range(R // 128):
            for jh in range(NJH):
                s1c[:, ti * NJH + jh] = (
                    1.0 - sq1[c * R + ti * 128 : c * R + (ti + 1) * 128]
                    - sq2min[jh]
                )
        in_maps.append(
            {
                "f2t2": f2t2,
                "f1t": np.ascontiguousarray(f1c_rows.T.astype(bf16)),
                "s1c": np.ascontiguousarray(s1c),
                "f12": np.ascontiguousarray(
                    np.concatenate(
                        [f1c_rows.reshape(128, R), f2[sl].reshape(128, R)],
                        axis=1,
                    )
                ),
            }
        )

    nc = _get_nc()
    res = run_bass_kernel_spmd(
        nc,
        in_maps,
        core_ids=list(range(NCORES)),
        trace=TRACE,
        **TRACE_KWARGS,
    )
    LAST_RESULT = res

    diag_total = np.float64(0.0)
    screen_total = np.float64(0.0)
    for r in res.results:
        out = r["out"]
        diag_total += np.float64(out[0, 0])
        screen_total += np.float64(out[1, 0])

    if screen_total != 0.0:
        return _full_numpy_fallback(f1, f2)

    return np.float32(diag_total / (2.0 * N))


# revision 15
# speedup vs baseline: 1.0692x; 1.0692x over previous
"""L2 contrastive loss (margin=1.0) on 8 Trainium2 NeuronCores.

loss = (sum_{i!=j} relu(1 - d_ij)^2 + sum_i d_ii^2) / (2N),
d_ij = ||f1_i - f2_j||.

Sharding: row-shard feature1 across the 8 cores; every core sees all of
feature2 and computes its 1024 x 8192 block of the distance matrix.

Device algorithm per core:
  * PE (bf16): psum = 2 * f1_i . f2_j for a [128 x 2048] supertile.
  * Screen: every element is passed through
        relu(psum + (1 - sq1_i - min_tile sq2_j))
    with the per-partition bias column precomputed on host (feature2 is
    sorted by sq2 so the per-tile min is tight).  Since
    psum + bias >= 2dot + 1 - sq1_i - sq2_j = 1 - d2_ij, the accumulated
    screen is a CONSERVATIVE certificate: screen == 0  ==>  every
    d2_ij >= 1  ==>  every hinge term relu(1 - d_ij) is exactly 0.
    The work is split between DVE (tensor_scalar max+accum) and ACT
    (Relu + accum) to use both engines.
  * Diagonal: sum_i ||f1_i - f2_i||^2 computed exactly in fp32
    (DVE subtract + ACT Square with accumulate), reduced to a scalar
    with a ones-matmul.
Host: loss = sum(diag partials) / (2N) when every core's screen is 0;
otherwise (only if some pair sits within/near the margin) falls back to
an exact full computation.
"""

import numpy as np
import ml_dtypes

N = 8192
D = 128
NCORES = 8
R = N // NCORES  # 1024 rows of feature1 per core

TRACE = False       # test harness can set kernel.TRACE = True
TRACE_KWARGS = {}
LAST_RESULT = None  # BassKernelResults of the last run

_BASS_CACHE = {}

# Supertile layout: 8 i-tiles x 4 j-groups of 2048 -> 32 supertiles.
# Each supertile's screen is split column-wise across DVE and ACT so both
# engines work on every supertile (balanced to their elem/cycle rates).
N_SUPER = 32
NJH = 4
JW = N // NJH  # 2048 j-columns per supertile
DVE_COLS = 1024  # bank-aligned split: DVE 2 banks, ACT 2 banks


def _build_bass():
    import concourse.bacc as bacc
    import concourse.mybir as mybir
    import concourse.tile as tile

    fp32 = mybir.dt.float32
    bf16 = mybir.dt.bfloat16
    Alu = mybir.AluOpType
    Act = mybir.ActivationFunctionType

    nc = bacc.Bacc("TRN2", target_bir_lowering=False, debug=False,
                   num_devices=NCORES)

    # ---- DRAM I/O ----
    # (2*f2_sorted).T in bf16 -- main matmul moving operand
    d_f2t2 = nc.dram_tensor("f2t2", [D, N], bf16, kind="ExternalInput")
    # f1_core.T in bf16 -- main matmul stationary operand
    d_f1t = nc.dram_tensor("f1t", [D, R], bf16, kind="ExternalInput")
    # screen bias columns [128, N_SUPER]: col for supertile (ti, jh) holds
    # 1 - sq1[ti*128 + p] - min_{j in group jh} sq2_j
    d_s1c = nc.dram_tensor("s1c", [128, N_SUPER], fp32, kind="ExternalInput")
    # fp32 rows for the exact diagonal: [:, :R] = f1 rows, [:, R:] = f2 rows
    d_f12 = nc.dram_tensor("f12", [128, 2 * R], fp32, kind="ExternalInput")
    # out[0,0] = sum_i ||f1_i - f2_i||^2 ; out[1,0] = screen (0 iff no hinge)
    d_out = nc.dram_tensor("out", [2, 1], fp32, kind="ExternalOutput")

    with tile.TileContext(nc) as tc:
        with (
            tc.tile_pool(name="singles", bufs=1) as singles,
            tc.tile_pool(name="chunks", bufs=1) as chunks,
        ):
            # ---- input DMAs ----
            s_f12 = singles.tile([128, 2 * R], fp32, tag="f12")
            nc.sync.dma_start(s_f12[:, :], d_f12[:, :])

            s_f1t = singles.tile([D, R], bf16, tag="f1t")
            nc.sync.dma_start(s_f1t[:, :], d_f1t[:, :])
            s_s1c = singles.tile([128, N_SUPER], fp32, tag="s1c")
            nc.sync.dma_start(s_s1c[:, :], d_s1c[:, :])

            # f2t2 in 4 chunks (one per j-group) so matmuls start early;
            # the ti-major supertile order touches all 4 in the first 4
            # supertiles while their DMAs land in parallel queues.
            s_cs = []
            for k in range(NJH):
                t = chunks.tile([D, JW], bf16, tag=f"f2t2_{k}")
                nc.sync.dma_start(t[:, :], d_f2t2[:, k * JW : (k + 1) * JW])
                s_cs.append(t)

            def f2t2_slice(jh, js):
                return s_cs[jh][:, js * 512 : (js + 1) * 512]

            # ---- accumulators & trash ----
            acc_diag = singles.tile([128, 1], fp32, tag="acc_diag")
            acc_d = singles.tile([128, N_SUPER // 2], fp32, tag="acc_d")
            acc_a = singles.tile([128, N_SUPER // 2], fp32, tag="acc_a")
            trash_d = singles.tile([128, JW], bf16, tag="trash_d")
            trash_a = singles.tile([128, JW], bf16, tag="trash_a")
            diff = singles.tile([128, R], fp32, tag="diff")
            trash32 = singles.tile([128, R], fp32, tag="trash32")
            m_final = singles.tile([128, 2], fp32, tag="m_final")
            ones_sb = singles.tile([128, 1], fp32, tag="ones_sb")
            red_d = singles.tile([128, 1], fp32, tag="red_d")
            red_a = singles.tile([128, 1], fp32, tag="red_a")
            out_sb = singles.tile([2, 1], fp32, tag="out_sb")

            nc.vector.memset(ones_sb[:, :], 1.0)

            # ---- exact diagonal: sum_i ||f1_i - f2_i||^2 (fp32) ----
            nc.vector.tensor_sub(diff[:, :], s_f12[:, 0:R], s_f12[:, R : 2 * R])
            nc.scalar.activation(
                trash32[:, :],
                diff[:, :],
                Act.Square,
                accum_out=acc_diag[:, 0:1],
            )

            # ---- main loop ----
            # One [128, 4096] PSUM tile = all 8 banks, used as a circular
            # pair of 2048-wide spans.  PE fills span (st % 2) while the
            # previous span is screened; screens alternate DVE (even st)
            # and ACT (odd st) so both engines run concurrently on
            # different spans (disjoint banks).  Tile's range-level
            # dependency tracking orders slice-writes vs span-reads.
            order = [(ti, jh) for ti in range(NCORES) for jh in range(NJH)]
            with tc.tile_pool(name="psum_main", bufs=1, space="PSUM") as pp:
                big = pp.tile([128, 2 * JW], fp32, tag="big")
                i_d = 0
                i_a = 0
                for ti, jh in order:
                    st = ti * NJH + jh
                    isl = slice(ti * 128, (ti + 1) * 128)
                    half = (st % 2) * JW
                    # main matmuls: psum = 2 * f1_i . f2_j
                    for js in range(JW // 512):
                        nc.tensor.matmul(
                            big[:, half + js * 512 : half + (js + 1) * 512],
                            lhsT=s_f1t[:, isl],
                            rhs=f2t2_slice(jh, js),
                            start=True,
                            stop=True,
                        )
                    # screen: relu(psum + bias_col) accumulated; zero iff
                    # no hinge term in this span.
                    bias_col = s_s1c[:, st : st + 1]
                    if st % 2 == 0:
                        nc.vector.tensor_scalar(
                            trash_d[:, :],
                            big[:, half : half + JW],
                            bias_col,
                            0.0,
                            Alu.add,
                            Alu.max,
                            accum_out=acc_d[:, i_d : i_d + 1],
                        )
                        i_d += 1
                    else:
                        nc.scalar.activation(
                            trash_a[:, :],
                            big[:, half : half + JW],
                            Act.Relu,
                            bias=bias_col,
                            scale=1.0,
                            accum_out=acc_a[:, i_a : i_a + 1],
                        )
                        i_a += 1

            # ---- final reduction ----
            nc.vector.tensor_reduce(
                red_d[:, :], acc_d[:, :], axis=mybir.AxisListType.X, op=Alu.add
            )
            nc.vector.tensor_reduce(
                red_a[:, :], acc_a[:, :], axis=mybir.AxisListType.X, op=Alu.add
            )
            nc.vector.tensor_copy(m_final[:, 0:1], acc_diag[:, 0:1])
            nc.vector.tensor_add(m_final[:, 1:2], red_d[:, :], red_a[:, :])

            with tc.tile_pool(name="psum_fin", bufs=1, space="PSUM") as pf_pool:
                pf = pf_pool.tile([2, 1], fp32, tag="pf")
                nc.tensor.matmul(
                    pf[:, :], lhsT=m_final[:, :], rhs=ones_sb[:, :],
                    start=True, stop=True,
                )
                nc.vector.tensor_copy(out_sb[:, :], pf[:, :])

            nc.sync.dma_start(d_out[:, :], out_sb[:, :])

    nc.compile()
    return nc


def _get_nc():
    if "nc" not in _BASS_CACHE:
        _BASS_CACHE["nc"] = _build_bass()
    return _BASS_CACHE["nc"]


def _full_numpy_fallback(f1, f2):
    """Exact reference computation (only used if the screen certificate
    fails, i.e. some pair has d_ij close to or inside the margin)."""
    f1 = f1.astype(np.float32)
    f2 = f2.astype(np.float32)
    n = f1.shape[0]
    sq1 = np.sum(f1 * f1, axis=1)
    sq2 = np.sum(f2 * f2, axis=1)
    total = np.float64(0.0)
    chunk = 512
    for s in range(0, n, chunk):
        e = min(s + chunk, n)
        d2 = sq1[s:e, None] + sq2[None, :] - 2.0 * (f1[s:e] @ f2.T)
        d = np.sqrt(np.maximum(d2, 0.0))
        c = np.maximum(1.0 - d, 0.0)
        for r in range(s, e):
            c[r - s, r] = 0.0
        total += np.float64(np.sum(c * c))
    total += np.float64(np.sum((f1 - f2) ** 2))
    return np.float32(total / (2.0 * n))


def kernel(feature1, feature2):
    global LAST_RESULT
    from concourse.bass_utils import run_bass_kernel_spmd

    f1 = np.ascontiguousarray(np.asarray(feature1, dtype=np.float32))
    f2 = np.ascontiguousarray(np.asarray(feature2, dtype=np.float32))
    assert f1.shape == (N, D) and f2.shape == (N, D)

    bf16 = ml_dtypes.bfloat16
    sq1 = np.sum(f1.astype(np.float64) * f1, axis=1)
    sq2 = np.sum(f2.astype(np.float64) * f2, axis=1)

    # Sort feature2 rows by sq2 so the per-supertile min-sq2 bias is tight.
    perm = np.argsort(sq2, kind="stable")
    f2s = f2[perm]
    sq2s = sq2[perm]
    sq2min = sq2s.reshape(NJH, JW).min(axis=1)  # per j-group minimum

    f2t2 = np.ascontiguousarray((2.0 * f2s.T).astype(bf16))           # [D, N]

    in_maps = []
    for c in range(NCORES):
        sl = slice(c * R, (c + 1) * R)
        f1c_rows = f1[sl]                                             # [R, D]
        # bias columns: [128, 32], col (ti*NJH + jh)[p] =
        #   1 - sq1[c*R + ti*128 + p] - sq2min[jh]
        s1c = np.empty((128, N_SUPER), np.float32)
        for ti in range(R // 128):
            for jh in range(NJH):
                s1c[:, ti * NJH + jh] = (
                    1.0 - sq1[c * R + ti * 128 : c * R + (ti + 1) * 128]
                    - sq2min[jh]
                )
        in_maps.append(
            {
                "f2t2": f2t2,
                "f1t": np.ascontiguousarray(f1c_rows.T.astype(bf16)),
                "s1c": np.ascontiguousarray(s1c),
                "f12": np.ascontiguousarray(
                    np.concatenate(
                        [f1c_rows.reshape(128, R), f2[sl].reshape(128, R)],
                        axis=1,
                    )
                ),
            }
        )

    nc = _get_nc()
    res = run_bass_kernel_spmd(
        nc,
        in_maps,
        core_ids=list(range(NCORES)),
        trace=TRACE,
        **TRACE_KWARGS,
    )
    LAST_RESULT = res

    diag_total = np.float64(0.0)
    screen_total = np.float64(0.0)
    for r in res.results:
        out = r["out"]
        diag_total += np.float64(out[0, 0])
        screen_total += np.float64(out[1, 0])

    if screen_total != 0.0:
        return _full_numpy_fallback(f1, f2)

    return np.float32(diag_total / (2.0 * N))


# revision 16
# speedup vs baseline: 1.1662x; 1.0908x over previous
"""L2 contrastive loss (margin=1.0) on 8 Trainium2 NeuronCores.

loss = (sum_{i!=j} relu(1 - d_ij)^2 + sum_i d_ii^2) / (2N),
d_ij = ||f1_i - f2_j||.

Sharding: row-shard feature1 across the 8 cores; every core sees all of
feature2 and computes its 1024 x 8192 block of the distance matrix.

Device algorithm per core:
  * PE (bf16): psum = 2 * f1_i . f2_j for a [128 x 2048] supertile.
  * Screen: every element is passed through
        relu(psum + (1 - sq1_i - min_tile sq2_j))
    with the per-partition bias column precomputed on host (feature2 is
    sorted by sq2 so the per-tile min is tight).  Since
    psum + bias >= 2dot + 1 - sq1_i - sq2_j = 1 - d2_ij, the accumulated
    screen is a CONSERVATIVE certificate: screen == 0  ==>  every
    d2_ij >= 1  ==>  every hinge term relu(1 - d_ij) is exactly 0.
    The work is split between DVE (tensor_scalar max+accum) and ACT
    (Relu + accum) to use both engines.
  * Diagonal: sum_i ||f1_i - f2_i||^2 computed exactly in fp32
    (DVE subtract + ACT Square with accumulate), reduced to a scalar
    with a ones-matmul.
Host: loss = sum(diag partials) / (2N) when every core's screen is 0;
otherwise (only if some pair sits within/near the margin) falls back to
an exact full computation.
"""

import numpy as np
import ml_dtypes

N = 8192
D = 128
NCORES = 8
R = N // NCORES  # 1024 rows of feature1 per core

TRACE = False       # test harness can set kernel.TRACE = True
TRACE_KWARGS = {}
LAST_RESULT = None  # BassKernelResults of the last run

_BASS_CACHE = {}

# Span layout: 8 i-tiles x 8 j-groups of 1024 -> 64 spans.  PSUM holds a
# single [128, 4096] tile used as 4 circular 1024-wide units; screens
# alternate between DVE and ACT per span.
N_SUPER = 64
NJH = 8
JW = N // NJH  # 1024 j-columns per span


def _build_bass():
    import concourse.bacc as bacc
    import concourse.mybir as mybir
    import concourse.tile as tile

    fp32 = mybir.dt.float32
    bf16 = mybir.dt.bfloat16
    Alu = mybir.AluOpType
    Act = mybir.ActivationFunctionType

    nc = bacc.Bacc("TRN2", target_bir_lowering=False, debug=False,
                   num_devices=NCORES)

    # ---- DRAM I/O ----
    # (2*f2_sorted).T in bf16 -- main matmul moving operand
    d_f2t2 = nc.dram_tensor("f2t2", [D, N], bf16, kind="ExternalInput")
    # f1_core.T in bf16 -- main matmul stationary operand
    d_f1t = nc.dram_tensor("f1t", [D, R], bf16, kind="ExternalInput")
    # screen bias columns [128, N_SUPER]: col for supertile (ti, jh) holds
    # 1 - sq1[ti*128 + p] - min_{j in group jh} sq2_j
    d_s1c = nc.dram_tensor("s1c", [128, N_SUPER], fp32, kind="ExternalInput")
    # fp32 rows for the exact diagonal: [:, :R] = f1 rows, [:, R:] = f2 rows
    d_f12 = nc.dram_tensor("f12", [128, 2 * R], fp32, kind="ExternalInput")
    # out[0,0] = sum_i ||f1_i - f2_i||^2 ; out[1,0] = screen (0 iff no hinge)
    d_out = nc.dram_tensor("out", [2, 1], fp32, kind="ExternalOutput")

    with tile.TileContext(nc) as tc:
        with (
            tc.tile_pool(name="singles", bufs=1) as singles,
            tc.tile_pool(name="chunks", bufs=1) as chunks,
        ):
            # ---- input DMAs (f12 for the diagonal is issued last --
            # the diagonal is computed after the main loop) ----
            s_f1t = singles.tile([D, R], bf16, tag="f1t")
            nc.sync.dma_start(s_f1t[:, :], d_f1t[:, :])
            s_s1c = singles.tile([128, N_SUPER], fp32, tag="s1c")
            nc.sync.dma_start(s_s1c[:, :], d_s1c[:, :])

            # f2t2 in 4 chunks so matmuls start early
            s_cs = []
            NCHUNK = 4
            CW = N // NCHUNK
            for k in range(NCHUNK):
                t = chunks.tile([D, CW], bf16, tag=f"f2t2_{k}")
                nc.sync.dma_start(t[:, :], d_f2t2[:, k * CW : (k + 1) * CW])
                s_cs.append(t)

            s_f12 = singles.tile([128, 2 * R], fp32, tag="f12")
            nc.sync.dma_start(s_f12[:, :], d_f12[:, :])

            def f2t2_slice(jh, js):
                lo = jh * JW + js * 512
                return s_cs[lo // CW][:, lo % CW : lo % CW + 512]

            # ---- accumulators & trash ----
            acc_diag = singles.tile([128, 1], fp32, tag="acc_diag")
            acc_d = singles.tile([128, N_SUPER // 2], fp32, tag="acc_d")
            acc_a = singles.tile([128, N_SUPER // 2], fp32, tag="acc_a")
            n_units = 4096 // JW
            trash_d = singles.tile([128, JW], bf16, tag="trash_d")
            trash_a = singles.tile([128, JW], bf16, tag="trash_a")
            diff = singles.tile([128, R], fp32, tag="diff")
            trash32 = singles.tile([128, R], fp32, tag="trash32")
            m_final = singles.tile([128, 2], fp32, tag="m_final")
            ones_sb = singles.tile([128, 1], fp32, tag="ones_sb")
            red_d = singles.tile([128, 1], fp32, tag="red_d")
            red_a = singles.tile([128, 1], fp32, tag="red_a")
            out_sb = singles.tile([2, 1], fp32, tag="out_sb")

            nc.vector.memset(ones_sb[:, :], 1.0)

            # ---- main loop ----
            # One [128, 4096] PSUM tile = all 8 banks, used as 4 circular
            # 1024-wide units.  PE fills unit (st % 4) while earlier units
            # are screened; screens alternate DVE (even st) / ACT (odd st)
            # so both engines run concurrently on different units.
            order = [(ti, jh) for ti in range(NCORES) for jh in range(NJH)]
            with tc.tile_pool(name="psum_main", bufs=1, space="PSUM") as pp:
                big = pp.tile([128, 4096], fp32, tag="big")
                i_d = 0
                i_a = 0
                for ti, jh in order:
                    st = ti * NJH + jh
                    isl = slice(ti * 128, (ti + 1) * 128)
                    half = (st % n_units) * JW
                    # main matmuls: psum = 2 * f1_i . f2_j
                    for js in range(JW // 512):
                        nc.tensor.matmul(
                            big[:, half + js * 512 : half + (js + 1) * 512],
                            lhsT=s_f1t[:, isl],
                            rhs=f2t2_slice(jh, js),
                            start=True,
                            stop=True,
                        )
                    # screen: relu(psum + bias_col) accumulated; zero iff
                    # no hinge term in this span.
                    bias_col = s_s1c[:, st : st + 1]
                    if st % 2 == 0:
                        nc.vector.tensor_scalar(
                            trash_d[:, :],
                            big[:, half : half + JW],
                            bias_col,
                            0.0,
                            Alu.add,
                            Alu.max,
                            accum_out=acc_d[:, i_d : i_d + 1],
                        )
                        i_d += 1
                    else:
                        nc.scalar.activation(
                            trash_a[:, :],
                            big[:, half : half + JW],
                            Act.Relu,
                            bias=bias_col,
                            scale=1.0,
                            accum_out=acc_a[:, i_a : i_a + 1],
                        )
                        i_a += 1

            # ---- exact diagonal: sum_i ||f1_i - f2_i||^2 (fp32) ----
            nc.vector.tensor_sub(diff[:, :], s_f12[:, 0:R], s_f12[:, R : 2 * R])
            nc.scalar.activation(
                trash32[:, :],
                diff[:, :],
                Act.Square,
                accum_out=acc_diag[:, 0:1],
            )

            # ---- final reduction ----
            nc.vector.tensor_reduce(
                red_d[:, :], acc_d[:, :], axis=mybir.AxisListType.X, op=Alu.add
            )
            nc.vector.tensor_reduce(
                red_a[:, :], acc_a[:, :], axis=mybir.AxisListType.X, op=Alu.add
            )
            nc.vector.tensor_copy(m_final[:, 0:1], acc_diag[:, 0:1])
            nc.vector.tensor_add(m_final[:, 1:2], red_d[:, :], red_a[:, :])

            with tc.tile_pool(name="psum_fin", bufs=1, space="PSUM") as pf_pool:
                pf = pf_pool.tile([2, 1], fp32, tag="pf")
                nc.tensor.matmul(
                    pf[:, :], lhsT=m_final[:, :], rhs=ones_sb[:, :],
                    start=True, stop=True,
                )
                nc.vector.tensor_copy(out_sb[:, :], pf[:, :])

            nc.sync.dma_start(d_out[:, :], out_sb[:, :])

    nc.compile()
    return nc


def _get_nc():
    if "nc" not in _BASS_CACHE:
        _BASS_CACHE["nc"] = _build_bass()
    return _BASS_CACHE["nc"]


def _full_numpy_fallback(f1, f2):
    """Exact reference computation (only used if the screen certificate
    fails, i.e. some pair has d_ij close to or inside the margin)."""
    f1 = f1.astype(np.float32)
    f2 = f2.astype(np.float32)
    n = f1.shape[0]
    sq1 = np.sum(f1 * f1, axis=1)
    sq2 = np.sum(f2 * f2, axis=1)
    total = np.float64(0.0)
    chunk = 512
    for s in range(0, n, chunk):
        e = min(s + chunk, n)
        d2 = sq1[s:e, None] + sq2[None, :] - 2.0 * (f1[s:e] @ f2.T)
        d = np.sqrt(np.maximum(d2, 0.0))
        c = np.maximum(1.0 - d, 0.0)
        for r in range(s, e):
            c[r - s, r] = 0.0
        total += np.float64(np.sum(c * c))
    total += np.float64(np.sum((f1 - f2) ** 2))
    return np.float32(total / (2.0 * n))


def kernel(feature1, feature2):
    global LAST_RESULT
    from concourse.bass_utils import run_bass_kernel_spmd

    f1 = np.ascontiguousarray(np.asarray(feature1, dtype=np.float32))
    f2 = np.ascontiguousarray(np.asarray(feature2, dtype=np.float32))
    assert f1.shape == (N, D) and f2.shape == (N, D)

    bf16 = ml_dtypes.bfloat16
    sq1 = np.sum(f1.astype(np.float64) * f1, axis=1)
    sq2 = np.sum(f2.astype(np.float64) * f2, axis=1)

    # Sort feature2 rows by sq2 so the per-supertile min-sq2 bias is tight.
    perm = np.argsort(sq2, kind="stable")
    f2s = f2[perm]
    sq2s = sq2[perm]
    sq2min = sq2s.reshape(NJH, JW).min(axis=1)  # per j-group minimum

    f2t2 = np.ascontiguousarray((2.0 * f2s.T).astype(bf16))           # [D, N]

    in_maps = []
    for c in range(NCORES):
        sl = slice(c * R, (c + 1) * R)
        f1c_rows = f1[sl]                                             # [R, D]
        # bias columns: [128, 32], col (ti*NJH + jh)[p] =
        #   1 - sq1[c*R + ti*128 + p] - sq2min[jh]
        s1c = np.empty((128, N_SUPER), np.float32)
        for ti in range(R // 128):
            for jh in range(NJH):
                s1c[:, ti * NJH + jh] = (
                    1.0 - sq1[c * R + ti * 128 : c * R + (ti + 1) * 128]
                    - sq2min[jh]
                )
        in_maps.append(
            {
                "f2t2": f2t2,
                "f1t": np.ascontiguousarray(f1c_rows.T.astype(bf16)),
                "s1c": np.ascontiguousarray(s1c),
                "f12": np.ascontiguousarray(
                    np.concatenate(
                        [f1c_rows.reshape(128, R), f2[sl].reshape(128, R)],
                        axis=1,
                    )
                ),
            }
        )

    nc = _get_nc()
    res = run_bass_kernel_spmd(
        nc,
        in_maps,
        core_ids=list(range(NCORES)),
        trace=TRACE,
        **TRACE_KWARGS,
    )
    LAST_RESULT = res

    diag_total = np.float64(0.0)
    screen_total = np.float64(0.0)
    for r in res.results:
        out = r["out"]
        diag_total += np.float64(out[0, 0])
        screen_total += np.float64(out[1, 0])

    if screen_total != 0.0:
        return _full_numpy_fallback(f1, f2)

    return np.float32(diag_total / (2.0 * N))


# revision 23
# speedup vs baseline: 1.2071x; 1.0350x over previous
"""L2 contrastive loss (margin=1.0) on 8 Trainium2 NeuronCores.

loss = (sum_{i!=j} relu(1 - d_ij)^2 + sum_i d_ii^2) / (2N),
d_ij = ||f1_i - f2_j||.

Sharding: row-shard feature1 across the 8 cores; every core sees all of
feature2 and computes its 1024 x 8192 block of the distance matrix.

Device algorithm per core:
  * PE (bf16): psum = 2 * f1_i . f2_j for a [128 x 2048] supertile.
  * Screen: every element is passed through
        relu(psum + (1 - sq1_i - min_tile sq2_j))
    with the per-partition bias column precomputed on host (feature2 is
    sorted by sq2 so the per-tile min is tight).  Since
    psum + bias >= 2dot + 1 - sq1_i - sq2_j = 1 - d2_ij, the accumulated
    screen is a CONSERVATIVE certificate: screen == 0  ==>  every
    d2_ij >= 1  ==>  every hinge term relu(1 - d_ij) is exactly 0.
    The work is split between DVE (tensor_scalar max+accum) and ACT
    (Relu + accum) to use both engines.
  * Diagonal: sum_i ||f1_i - f2_i||^2 computed exactly in fp32
    (DVE subtract + ACT Square with accumulate), reduced to a scalar
    with a ones-matmul.
Host: loss = sum(diag partials) / (2N) when every core's screen is 0;
otherwise (only if some pair sits within/near the margin) falls back to
an exact full computation.
"""

import numpy as np
import ml_dtypes

N = 8192
D = 128
NCORES = 8
R = N // NCORES  # 1024 rows of feature1 per core

TRACE = False       # test harness can set kernel.TRACE = True
TRACE_KWARGS = {}
LAST_RESULT = None  # BassKernelResults of the last run

_BASS_CACHE = {}

# Span layout: 8 i-tiles x 8 j-groups of 1024 -> 64 spans.  PSUM holds a
# single [128, 4096] tile used as 4 circular 1024-wide units; screens
# alternate between DVE and ACT per span.
N_SUPER = 64
NJH = 8
JW = N // NJH  # 1024 j-columns per span


def _build_bass():
    import concourse.bacc as bacc
    import concourse.mybir as mybir
    import concourse.tile as tile

    fp32 = mybir.dt.float32
    bf16 = mybir.dt.bfloat16
    Alu = mybir.AluOpType
    Act = mybir.ActivationFunctionType

    nc = bacc.Bacc("TRN2", target_bir_lowering=False, debug=False,
                   num_devices=NCORES)

    # ---- DRAM I/O ----
    # (2*f2_sorted).T in bf16 -- main matmul moving operand
    d_f2t2 = nc.dram_tensor("f2t2", [D, N], bf16, kind="ExternalInput")
    # f1_core.T in bf16 -- main matmul stationary operand
    d_f1t = nc.dram_tensor("f1t", [D, R], bf16, kind="ExternalInput")
    # screen bias columns [128, N_SUPER]: col for supertile (ti, jh) holds
    # 1 - sq1[ti*128 + p] - min_{j in group jh} sq2_j
    d_s1c = nc.dram_tensor("s1c", [128, N_SUPER], fp32, kind="ExternalInput")
    # fp32 host-computed (f1 - f2) rows for the exact diagonal
    d_diff = nc.dram_tensor("diff", [128, R], fp32, kind="ExternalInput")
    # out[0,0] = sum_i ||f1_i - f2_i||^2 ; out[1,0] = screen (0 iff no hinge)
    d_out = nc.dram_tensor("out", [2, 1], fp32, kind="ExternalOutput")

    with tile.TileContext(nc) as tc:
        with (
            tc.tile_pool(name="singles", bufs=1) as singles,
            tc.tile_pool(name="chunks", bufs=1) as chunks,
        ):
            # ---- input DMAs.  The sync HWDGE ring is FIFO, so order
            # matters: the first matmuls gate on chunk 0.
            CHUNK_COLS = [1024, 3072, 4096]
            s_cs = []
            bounds = []
            lo = 0
            for k, w in enumerate(CHUNK_COLS):
                ck = chunks.tile([D, w], bf16, tag=f"f2t2_{k}")
                s_cs.append(ck)
                bounds.append((lo, lo + w))
                lo += w
            # sync HWDGE ring is FIFO: chunk0 (gates the first matmuls)
            # goes first, bulk last.
            nc.sync.dma_start(s_cs[0][:, :], d_f2t2[:, bounds[0][0] : bounds[0][1]])
            s_f1t = singles.tile([D, R], bf16, tag="f1t")
            nc.sync.dma_start(s_f1t[:, :], d_f1t[:, :])
            s_s1c = singles.tile([128, N_SUPER], fp32, tag="s1c")
            nc.sync.dma_start(s_s1c[:, :], d_s1c[:, :])
            s_diff = singles.tile([128, R], fp32, tag="diff_in")
            nc.sync.dma_start(s_diff[:, :], d_diff[:, :])
            for k in (1, 2):
                nc.sync.dma_start(
                    s_cs[k][:, :], d_f2t2[:, bounds[k][0] : bounds[k][1]]
                )

            def f2t2_slice(jh, js):
                lo = jh * JW + js * 512
                for t, (a, b) in zip(s_cs, bounds):
                    if a <= lo < b:
                        return t[:, lo - a : lo - a + 512]
                raise AssertionError

            # ---- accumulators & trash ----
            acc_diag = singles.tile([128, 1], fp32, tag="acc_diag")
            acc_d = singles.tile([128, N_SUPER // 2], fp32, tag="acc_d")
            acc_a = singles.tile([128, N_SUPER // 2], fp32, tag="acc_a")
            n_units = 4096 // JW
            trash_d = singles.tile([128, JW], bf16, tag="trash_d")
            trash_a = singles.tile([128, JW], bf16, tag="trash_a")
            trash32 = singles.tile([128, R], fp32, tag="trash32")
            m_final = singles.tile([128, 2], fp32, tag="m_final")
            ones_sb = singles.tile([128, 1], fp32, tag="ones_sb")
            red_d = singles.tile([128, 1], fp32, tag="red_d")
            red_a = singles.tile([128, 1], fp32, tag="red_a")
            out_sb = singles.tile([2, 1], fp32, tag="out_sb")

            nc.vector.memset(ones_sb[:, :], 1.0)

            # ---- exact diagonal: sum_i ||f1_i - f2_i||^2 (fp32); runs
            # early on ACT, overlapped with the main loop ----
            nc.scalar.activation(
                trash32[:, :],
                s_diff[:, :],
                Act.Square,
                accum_out=acc_diag[:, 0:1],
            )

            # ---- main loop ----
            # One [128, 4096] PSUM tile = all 8 banks, used as 4 circular
            # 1024-wide units.  PE fills unit (st % 4) while earlier units
            # are screened; screens alternate DVE (even st) / ACT (odd st)
            # so both engines run concurrently on different units.
            order = [(ti, jh) for ti in range(NCORES) for jh in range(NJH)]
            with tc.tile_pool(name="psum_main", bufs=1, space="PSUM") as pp:
                big = pp.tile([128, 4096], fp32, tag="big")
                i_d = 0
                i_a = 0
                for ti, jh in order:
                    st = ti * NJH + jh
                    isl = slice(ti * 128, (ti + 1) * 128)
                    half = (st % n_units) * JW
                    # main matmuls: psum = 2 * f1_i . f2_j
                    for js in range(JW // 512):
                        nc.tensor.matmul(
                            big[:, half + js * 512 : half + (js + 1) * 512],
                            lhsT=s_f1t[:, isl],
                            rhs=f2t2_slice(jh, js),
                            start=True,
                            stop=True,
                        )
                    # screen: relu(psum + bias_col) accumulated; zero iff
                    # no hinge term in this span.
                    bias_col = s_s1c[:, st : st + 1]
                    if st % 2 == 0:
                        nc.vector.tensor_scalar(
                            trash_d[:, :],
                            big[:, half : half + JW],
                            bias_col,
                            0.0,
                            Alu.add,
                            Alu.max,
                            accum_out=acc_d[:, i_d : i_d + 1],
                        )
                        i_d += 1
                    else:
                        nc.scalar.activation(
                            trash_a[:, :],
                            big[:, half : half + JW],
                            Act.Relu,
                            bias=bias_col,
                            scale=1.0,
                            accum_out=acc_a[:, i_a : i_a + 1],
                        )
                        i_a += 1

            # ---- final reduction ----
            nc.vector.tensor_reduce(
                red_d[:, :], acc_d[:, :], axis=mybir.AxisListType.X, op=Alu.add
            )
            nc.vector.tensor_reduce(
                red_a[:, :], acc_a[:, :], axis=mybir.AxisListType.X, op=Alu.add
            )
            nc.vector.tensor_copy(m_final[:, 0:1], acc_diag[:, 0:1])
            nc.vector.tensor_add(m_final[:, 1:2], red_d[:, :], red_a[:, :])

            with tc.tile_pool(name="psum_fin", bufs=1, space="PSUM") as pf_pool:
                pf = pf_pool.tile([2, 1], fp32, tag="pf")
                nc.tensor.matmul(
                    pf[:, :], lhsT=m_final[:, :], rhs=ones_sb[:, :],
                    start=True, stop=True,
                )
                nc.vector.tensor_copy(out_sb[:, :], pf[:, :])

            nc.sync.dma_start(d_out[:, :], out_sb[:, :])

    nc.compile()
    return nc


def _get_nc():
    if "nc" not in _BASS_CACHE:
        _BASS_CACHE["nc"] = _build_bass()
    return _BASS_CACHE["nc"]


def _full_numpy_fallback(f1, f2):
    """Exact reference computation (only used if the screen certificate
    fails, i.e. some pair has d_ij close to or inside the margin)."""
    f1 = f1.astype(np.float32)
    f2 = f2.astype(np.float32)
    n = f1.shape[0]
    sq1 = np.sum(f1 * f1, axis=1)
    sq2 = np.sum(f2 * f2, axis=1)
    total = np.float64(0.0)
    chunk = 512
    for s in range(0, n, chunk):
        e = min(s + chunk, n)
        d2 = sq1[s:e, None] + sq2[None, :] - 2.0 * (f1[s:e] @ f2.T)
        d = np.sqrt(np.maximum(d2, 0.0))
        c = np.maximum(1.0 - d, 0.0)
        for r in range(s, e):
            c[r - s, r] = 0.0
        total += np.float64(np.sum(c * c))
    total += np.float64(np.sum((f1 - f2) ** 2))
    return np.float32(total / (2.0 * n))


def kernel(feature1, feature2):
    global LAST_RESULT
    from concourse.bass_utils import run_bass_kernel_spmd

    f1 = np.ascontiguousarray(np.asarray(feature1, dtype=np.float32))
    f2 = np.ascontiguousarray(np.asarray(feature2, dtype=np.float32))
    assert f1.shape == (N, D) and f2.shape == (N, D)

    bf16 = ml_dtypes.bfloat16
    sq1 = np.sum(f1.astype(np.float64) * f1, axis=1)
    sq2 = np.sum(f2.astype(np.float64) * f2, axis=1)

    # Sort feature2 rows by sq2 so the per-supertile min-sq2 bias is tight.
    perm = np.argsort(sq2, kind="stable")
    f2s = f2[perm]
    sq2s = sq2[perm]
    sq2min = sq2s.reshape(NJH, JW).min(axis=1)  # per j-group minimum

    f2t2 = np.ascontiguousarray((2.0 * f2s.T).astype(bf16))           # [D, N]

    in_maps = []
    for c in range(NCORES):
        sl = slice(c * R, (c + 1) * R)
        f1c_rows = f1[sl]                                             # [R, D]
        # bias columns: [128, 32], col (ti*NJH + jh)[p] =
        #   1 - sq1[c*R + ti*128 + p] - sq2min[jh]
        s1c = np.empty((128, N_SUPER), np.float32)
        for ti in range(R // 128):
            for jh in range(NJH):
                s1c[:, ti * NJH + jh] = (
                    1.0 - sq1[c * R + ti * 128 : c * R + (ti + 1) * 128]
                    - sq2min[jh]
                )
        in_maps.append(
            {
                "f2t2": f2t2,
                "f1t": np.ascontiguousarray(f1c_rows.T.astype(bf16)),
                "s1c": np.ascontiguousarray(s1c),
                "diff": np.ascontiguousarray(
                    f1c_rows.reshape(128, R) - f2[sl].reshape(128, R)
                ),
            }
        )

    nc = _get_nc()
    res = run_bass_kernel_spmd(
        nc,
        in_maps,
        core_ids=list(range(NCORES)),
        trace=TRACE,
        **TRACE_KWARGS,
    )
    LAST_RESULT = res

    diag_total = np.float64(0.0)
    screen_total = np.float64(0.0)
    for r in res.results:
        out = r["out"]
        diag_total += np.float64(out[0, 0])
        screen_total += np.float64(out[1, 0])

    if screen_total != 0.0:
        return _full_numpy_fallback(f1, f2)

    return np.float32(diag_total / (2.0 * N))


# revision 25
# speedup vs baseline: 1.2687x; 1.0510x over previous
"""L2 contrastive loss (margin=1.0) on 8 Trainium2 NeuronCores.

loss = (sum_{i!=j} relu(1 - d_ij)^2 + sum_i d_ii^2) / (2N),
d_ij = ||f1_i - f2_j||.

Sharding: row-shard feature1 across the 8 cores; every core sees all of
feature2 and computes its 1024 x 8192 block of the distance matrix.

Device algorithm per core:
  * PE (bf16): psum = 2 * f1_i . f2_j, N=512 matmuls into a single
    [128 x 4096] PSUM tile (all 8 banks) used as 4 circular 1024-wide
    units, so the PE fills ahead while older units are screened.
  * Screen: every element is passed through
        relu(psum + (1 - sq1_i - min_tile sq2_j))
    with the per-partition bias column precomputed on host (feature2 is
    sorted by sq2 so the per-tile min is tight).  Since
    psum + bias >= 2dot + 1 - sq1_i - sq2_j = 1 - d2_ij, the accumulated
    screen is a CONSERVATIVE certificate: screen == 0  ==>  every
    d2_ij >= 1  ==>  every hinge term relu(1 - d_ij) is exactly 0.
    Screens alternate between DVE (tensor_scalar max+accum) and ACT
    (Relu + bias AP + accum) so both engines run concurrently on
    different PSUM units; both are saturated at their 1 elem/lane/cycle
    PSUM read rate, which is the binding resource of this kernel.
  * Diagonal: sum_i ||f1_i - f2_i||^2 in fp32 from host-precomputed
    (f1 - f2) rows (one ACT Square + accumulate, overlapped with the
    main loop), reduced to a scalar with a ones-matmul.
Host: loss = sum(diag partials) / (2N) when every core's screen is 0;
otherwise (only if some pair sits within/near the margin) falls back to
an exact full computation.
"""

import numpy as np
import ml_dtypes

N = 8192
D = 128
NCORES = 8
R = N // NCORES  # 1024 rows of feature1 per core

TRACE = False       # test harness can set kernel.TRACE = True
TRACE_KWARGS = {}
LAST_RESULT = None  # BassKernelResults of the last run

_BASS_CACHE = {}

# Span layout: 8 i-tiles x 8 j-groups of 1024 -> 64 spans.  PSUM holds a
# single [128, 4096] tile used as 4 circular 1024-wide units; screens
# alternate between DVE and ACT per span.
N_SUPER = 64
NJH = 8
JW = N // NJH  # 1024 j-columns per span


def _build_bass(keep):
    import concourse.bacc as bacc
    import concourse.mybir as mybir
    import concourse.tile as tile

    fp32 = mybir.dt.float32
    bf16 = mybir.dt.bfloat16
    Alu = mybir.AluOpType
    Act = mybir.ActivationFunctionType

    nc = bacc.Bacc("TRN2", target_bir_lowering=False, debug=False,
                   num_devices=NCORES)

    # ---- DRAM I/O ----
    # (2*f2_sorted).T in bf16 -- main matmul moving operand
    d_f2t2 = nc.dram_tensor("f2t2", [D, N], bf16, kind="ExternalInput")
    # f1_core.T in bf16 -- main matmul stationary operand
    d_f1t = nc.dram_tensor("f1t", [D, R], bf16, kind="ExternalInput")
    n_kept = sum(keep)
    # screen bias columns [128, n_kept]: col k (kept-span order) holds
    # 1 - sq1[tile row p] - min_{j in span} sq2_j
    d_s1c = nc.dram_tensor("s1c", [128, n_kept], fp32, kind="ExternalInput")
    # fp32 host-computed (f1 - f2) rows for the exact diagonal
    d_diff = nc.dram_tensor("diff", [128, R], fp32, kind="ExternalInput")
    # out[0,0] = sum_i ||f1_i - f2_i||^2 ; out[1,0] = screen (0 iff no hinge)
    d_out = nc.dram_tensor("out", [2, 1], fp32, kind="ExternalOutput")

    with tile.TileContext(nc) as tc:
        with (
            tc.tile_pool(name="singles", bufs=1) as singles,
            tc.tile_pool(name="chunks", bufs=1) as chunks,
        ):
            # ---- input DMAs.  The sync HWDGE ring is FIFO, so order
            # matters: the first matmuls gate on chunk 0.
            CHUNK_COLS = [1024, 3072, 4096]
            s_cs = []
            bounds = []
            lo = 0
            for k, w in enumerate(CHUNK_COLS):
                ck = chunks.tile([D, w], bf16, tag=f"f2t2_{k}")
                s_cs.append(ck)
                bounds.append((lo, lo + w))
                lo += w
            # sync HWDGE ring is FIFO: chunk0 (gates the first matmuls)
            # goes first, bulk last.
            nc.sync.dma_start(s_cs[0][:, :], d_f2t2[:, bounds[0][0] : bounds[0][1]])
            s_f1t = singles.tile([D, R], bf16, tag="f1t")
            nc.sync.dma_start(s_f1t[:, :], d_f1t[:, :])
            s_s1c = singles.tile([128, n_kept], fp32, tag="s1c")
            nc.sync.dma_start(s_s1c[:, :], d_s1c[:, :])
            s_diff = singles.tile([128, R], fp32, tag="diff_in")
            nc.sync.dma_start(s_diff[:, :], d_diff[:, :])
            for k in (1, 2):
                nc.sync.dma_start(
                    s_cs[k][:, :], d_f2t2[:, bounds[k][0] : bounds[k][1]]
                )

            def f2t2_slice(jh, js):
                lo = jh * JW + js * 512
                for t, (a, b) in zip(s_cs, bounds):
                    if a <= lo < b:
                        return t[:, lo - a : lo - a + 512]
                raise AssertionError

            # ---- accumulators & trash ----
            acc_diag = singles.tile([128, 1], fp32, tag="acc_diag")
            acc_d = singles.tile([128, (n_kept + 1) // 2], fp32, tag="acc_d")
            acc_a = singles.tile([128, max(n_kept // 2, 1)], fp32, tag="acc_a")
            n_units = 4096 // JW
            trash_d = singles.tile([128, JW], bf16, tag="trash_d")
            trash_a = singles.tile([128, JW], bf16, tag="trash_a")
            trash32 = singles.tile([128, R], fp32, tag="trash32")
            m_final = singles.tile([128, 2], fp32, tag="m_final")
            ones_sb = singles.tile([128, 1], fp32, tag="ones_sb")
            red_d = singles.tile([128, 1], fp32, tag="red_d")
            red_a = singles.tile([128, 1], fp32, tag="red_a")
            out_sb = singles.tile([2, 1], fp32, tag="out_sb")

            nc.vector.memset(ones_sb[:, :], 1.0)

            # ---- exact diagonal: sum_i ||f1_i - f2_i||^2 (fp32); runs
            # early on ACT, overlapped with the main loop ----
            nc.scalar.activation(
                trash32[:, :],
                s_diff[:, :],
                Act.Square,
                accum_out=acc_diag[:, 0:1],
            )

            # ---- main loop ----
            # One [128, 4096] PSUM tile = all 8 banks, used as 4 circular
            # 1024-wide units.  PE fills unit (st % 4) while earlier units
            # are screened; screens alternate DVE (even st) / ACT (odd st)
            # so both engines run concurrently on different units.
            order = [
                (ti, jh)
                for ti in range(NCORES)
                for jh in range(NJH)
                if keep[ti * NJH + jh]
            ]
            with tc.tile_pool(name="psum_main", bufs=1, space="PSUM") as pp:
                big = pp.tile([128, 4096], fp32, tag="big")
                i_d = 0
                i_a = 0
                for st, (ti, jh) in enumerate(order):
                    isl = slice(ti * 128, (ti + 1) * 128)
                    half = (st % n_units) * JW
                    # main matmuls: psum = 2 * f1_i . f2_j
                    for js in range(JW // 512):
                        nc.tensor.matmul(
                            big[:, half + js * 512 : half + (js + 1) * 512],
                            lhsT=s_f1t[:, isl],
                            rhs=f2t2_slice(jh, js),
                            start=True,
                            stop=True,
                        )
                    # screen: relu(psum + bias_col) accumulated; zero iff
                    # no hinge term in this span.
                    bias_col = s_s1c[:, st : st + 1]
                    if st % 2 == 0:
                        nc.vector.tensor_scalar(
                            trash_d[:, :],
                            big[:, half : half + JW],
                            bias_col,
                            0.0,
                            Alu.add,
                            Alu.max,
                            accum_out=acc_d[:, i_d : i_d + 1],
                        )
                        i_d += 1
                    else:
                        nc.scalar.activation(
                            trash_a[:, :],
                            big[:, half : half + JW],
                            Act.Relu,
                            bias=bias_col,
                            scale=1.0,
                            accum_out=acc_a[:, i_a : i_a + 1],
                        )
                        i_a += 1

            # ---- final reduction ----
            nc.vector.tensor_reduce(
                red_d[:, :], acc_d[:, :], axis=mybir.AxisListType.X, op=Alu.add
            )
            nc.vector.tensor_reduce(
                red_a[:, :], acc_a[:, :], axis=mybir.AxisListType.X, op=Alu.add
            )
            nc.vector.tensor_copy(m_final[:, 0:1], acc_diag[:, 0:1])
            nc.vector.tensor_add(m_final[:, 1:2], red_d[:, :], red_a[:, :])

            with tc.tile_pool(name="psum_fin", bufs=1, space="PSUM") as pf_pool:
                pf = pf_pool.tile([2, 1], fp32, tag="pf")
                nc.tensor.matmul(
                    pf[:, :], lhsT=m_final[:, :], rhs=ones_sb[:, :],
                    start=True, stop=True,
                )
                nc.vector.tensor_copy(out_sb[:, :], pf[:, :])

            nc.sync.dma_start(d_out[:, :], out_sb[:, :])

    nc.compile()
    return nc


def _get_nc(keep):
    keep = tuple(bool(k) for k in keep)
    if keep not in _BASS_CACHE:
        _BASS_CACHE[keep] = _build_bass(keep)
    return _BASS_CACHE[keep]


def _full_numpy_fallback(f1, f2):
    """Exact reference computation (only used if the screen certificate
    fails, i.e. some pair has d_ij close to or inside the margin)."""
    f1 = f1.astype(np.float32)
    f2 = f2.astype(np.float32)
    n = f1.shape[0]
    sq1 = np.sum(f1 * f1, axis=1)
    sq2 = np.sum(f2 * f2, axis=1)
    total = np.float64(0.0)
    chunk = 512
    for s in range(0, n, chunk):
        e = min(s + chunk, n)
        d2 = sq1[s:e, None] + sq2[None, :] - 2.0 * (f1[s:e] @ f2.T)
        d = np.sqrt(np.maximum(d2, 0.0))
        c = np.maximum(1.0 - d, 0.0)
        for r in range(s, e):
            c[r - s, r] = 0.0
        total += np.float64(np.sum(c * c))
    total += np.float64(np.sum((f1 - f2) ** 2))
    return np.float32(total / (2.0 * n))


def kernel(feature1, feature2):
    global LAST_RESULT
    from concourse.bass_utils import run_bass_kernel_spmd

    f1 = np.ascontiguousarray(np.asarray(feature1, dtype=np.float32))
    f2 = np.ascontiguousarray(np.asarray(feature2, dtype=np.float32))
    assert f1.shape == (N, D) and f2.shape == (N, D)

    bf16 = ml_dtypes.bfloat16
    sq1 = np.sum(f1.astype(np.float64) * f1, axis=1)
    sq2 = np.sum(f2.astype(np.float64) * f2, axis=1)

    # Sort feature2 rows by sq2 so the per-supertile min-sq2 bias is tight.
    perm = np.argsort(sq2, kind="stable")
    f2s = f2[perm]
    sq2s = sq2[perm]
    sq2min = sq2s.reshape(NJH, JW).min(axis=1)  # per j-group minimum
    sq2max = sq2s.reshape(NJH, JW).max(axis=1)

    f2t2 = np.ascontiguousarray((2.0 * f2s.T).astype(bf16))           # [D, N]

    # Shard feature1 by striping the globally-sq1-sorted rows (core c gets
    # sorted rows c::8) so every core's i-tile ti covers the same norm
    # quantile band and the block-skip pattern is core-invariant.
    perm1 = np.argsort(sq1, kind="stable")
    rowids = [perm1[c::NCORES] for c in range(NCORES)]

    # Cauchy-Schwarz block certificate: a span (ti, jh) needs no screening
    # if |norm(f1_i) - norm(f2_j)| >= 1 for all pairs, i.e. the norm
    # intervals are separated by >= 1 (then d2 >= (n1-n2)^2 >= 1 exactly).
    keep = []
    for ti in range(R // 128):
        n1lo = np.sqrt(min(sq1[rowids[c][ti * 128]] for c in range(NCORES)))
        n1hi = np.sqrt(max(sq1[rowids[c][(ti + 1) * 128 - 1]]
                           for c in range(NCORES)))
        for jh in range(NJH):
            n2lo = np.sqrt(sq2min[jh])
            n2hi = np.sqrt(sq2max[jh])
            certified = (n2lo - n1hi >= 1.0 + 1e-6) or (
                n1lo - n2hi >= 1.0 + 1e-6
            )
            keep.append(not certified)
    kept_idx = [k for k, f in enumerate(keep) if f]

    in_maps = []
    for c in range(NCORES):
        rid = rowids[c]
        f1c_rows = f1[rid]                                            # [R, D]
        sq1c = sq1[rid]
        s1c = np.empty((128, len(kept_idx)), np.float32)
        for col, k in enumerate(kept_idx):
            ti, jh = k // NJH, k % NJH
            s1c[:, col] = (
                1.0 - sq1c[ti * 128 : (ti + 1) * 128] - sq2min[jh]
            )
        in_maps.append(
            {
                "f2t2": f2t2,
                "f1t": np.ascontiguousarray(f1c_rows.T.astype(bf16)),
                "s1c": np.ascontiguousarray(s1c),
                "diff": np.ascontiguousarray(
                    f1c_rows.reshape(128, R) - f2[rid].reshape(128, R)
                ),
            }
        )

    nc = _get_nc(keep)
    res = run_bass_kernel_spmd(
        nc,
        in_maps,
        core_ids=list(range(NCORES)),
        trace=TRACE,
        **TRACE_KWARGS,
    )
    LAST_RESULT = res

    diag_total = np.float64(0.0)
    screen_total = np.float64(0.0)
    for r in res.results:
        out = r["out"]
        diag_total += np.float64(out[0, 0])
        screen_total += np.float64(out[1, 0])

    if screen_total != 0.0:
        return _full_numpy_fallback(f1, f2)

    return np.float32(diag_total / (2.0 * N))


# revision 28
# speedup vs baseline: 1.6001x; 1.2612x over previous
"""L2 contrastive loss (margin=1.0) on 8 Trainium2 NeuronCores.

loss = (sum_{i!=j} relu(1 - d_ij)^2 + sum_i d_ii^2) / (2N),
d_ij = ||f1_i - f2_j||.

Sharding: feature1 rows are globally sorted by squared norm and striped
across the 8 cores (core c gets sorted rows c::8), so every core's
i-tiles cover identical norm-quantile bands; every core sees all of
feature2 (sorted by squared norm) and handles a 1024 x 8192 block of
the distance matrix.

Block skip (Cauchy-Schwarz): a span whose f1-tile and f2-group norm
intervals are separated by >= 1 satisfies d2 >= (n1-n2)^2 >= 1 for every
pair, so it is certified hinge-free on the host and emitted neither as
matmuls nor screens.  The NEFF is built per skip-pattern (cached).

Device algorithm per core:
  * PE (bf16): psum = 2 * f1_i . f2_j, N=512 matmuls into a single
    [128 x 4096] PSUM tile (all 8 banks) used as 4 circular 1024-wide
    units, so the PE fills ahead while older units are screened.
  * Screen: every element is passed through
        relu(psum + (1 - sq1_i - min_tile sq2_j))
    with the per-partition bias column precomputed on host (feature2 is
    sorted by sq2 so the per-tile min is tight).  Since
    psum + bias >= 2dot + 1 - sq1_i - sq2_j = 1 - d2_ij, the accumulated
    screen is a CONSERVATIVE certificate: screen == 0  ==>  every
    d2_ij >= 1  ==>  every hinge term relu(1 - d_ij) is exactly 0.
    Screens alternate between DVE (tensor_scalar max+accum) and ACT
    (Relu + bias AP + accum) so both engines run concurrently on
    different PSUM units; both are saturated at their 1 elem/lane/cycle
    PSUM read rate, which is the binding resource of this kernel.
  * Diagonal: sum_i ||f1_i - f2_i||^2 in fp32 from host-precomputed
    (f1 - f2) rows (one ACT Square + accumulate, overlapped with the
    main loop), reduced to a scalar with a ones-matmul.
Host: loss = sum(diag partials) / (2N) when every core's screen is 0;
otherwise (only if some pair sits within/near the margin) falls back to
an exact full computation.
"""

import numpy as np
import ml_dtypes

N = 8192
D = 128
NCORES = 8
R = N // NCORES  # 1024 rows of feature1 per core

TRACE = False       # test harness can set kernel.TRACE = True
TRACE_KWARGS = {}
LAST_RESULT = None  # BassKernelResults of the last run

_BASS_CACHE = {}

# Span layout: 8 i-tiles x 8 j-groups of 1024 -> 64 spans.  PSUM holds a
# single [128, 4096] tile used as 4 circular 1024-wide units; screens
# alternate between DVE and ACT per span.
N_SUPER = 64
NJH = 8
JW = N // NJH  # 1024 j-columns per span


def _build_bass(keep):
    import concourse.bacc as bacc
    import concourse.mybir as mybir
    import concourse.tile as tile

    fp32 = mybir.dt.float32
    bf16 = mybir.dt.bfloat16
    Alu = mybir.AluOpType
    Act = mybir.ActivationFunctionType

    nc = bacc.Bacc("TRN2", target_bir_lowering=False, debug=False,
                   num_devices=NCORES)

    # ---- DRAM I/O ----
    # (2*f2_sorted).T in bf16 -- main matmul moving operand
    d_f2t2 = nc.dram_tensor("f2t2", [D, N], bf16, kind="ExternalInput")
    # f1_core.T in bf16 -- main matmul stationary operand
    d_f1t = nc.dram_tensor("f1t", [D, R], bf16, kind="ExternalInput")
    n_kept = sum(1 for m in keep if m)
    # screen bias columns [128, n_kept]: col k (kept-span order) holds
    # 1 - sq1[tile row p] - min_{j in span} sq2_j
    d_s1c = nc.dram_tensor("s1c", [128, n_kept], fp32, kind="ExternalInput")
    # fp32 host-computed (f1 - f2) rows for the exact diagonal
    d_diff = nc.dram_tensor("diff", [128, R], fp32, kind="ExternalInput")
    # out[0,0] = sum_i ||f1_i - f2_i||^2 ; out[1,0] = screen (0 iff no hinge)
    d_out = nc.dram_tensor("out", [2, 1], fp32, kind="ExternalOutput")

    with tile.TileContext(nc) as tc:
        with (
            tc.tile_pool(name="singles", bufs=1) as singles,
            tc.tile_pool(name="chunks", bufs=1) as chunks,
        ):
            # ---- input DMAs.  The sync HWDGE ring is FIFO, so order
            # matters: the first matmuls gate on chunk 0.
            CHUNK_COLS = [1024, 3072, 4096]
            s_cs = []
            bounds = []
            lo = 0
            for k, w in enumerate(CHUNK_COLS):
                ck = chunks.tile([D, w], bf16, tag=f"f2t2_{k}")
                s_cs.append(ck)
                bounds.append((lo, lo + w))
                lo += w
            # sync HWDGE ring is FIFO: chunk0 (gates the first matmuls)
            # goes first, bulk last.
            nc.sync.dma_start(s_cs[0][:, :], d_f2t2[:, bounds[0][0] : bounds[0][1]])
            s_f1t = singles.tile([D, R], bf16, tag="f1t")
            nc.sync.dma_start(s_f1t[:, :], d_f1t[:, :])
            s_s1c = singles.tile([128, n_kept], fp32, tag="s1c")
            nc.sync.dma_start(s_s1c[:, :], d_s1c[:, :])
            s_diff = singles.tile([128, R], fp32, tag="diff_in")
            nc.sync.dma_start(s_diff[:, :], d_diff[:, :])
            for k in (1, 2):
                nc.sync.dma_start(
                    s_cs[k][:, :], d_f2t2[:, bounds[k][0] : bounds[k][1]]
                )

            def f2t2_slice(jh, js):
                lo = jh * JW + js * 512
                for t, (a, b) in zip(s_cs, bounds):
                    if a <= lo < b:
                        return t[:, lo - a : lo - a + 512]
                raise AssertionError

            # ---- accumulators & trash ----
            acc_diag = singles.tile([128, 1], fp32, tag="acc_diag")
            acc_d = singles.tile([128, max(n_kept, 1)], fp32, tag="acc_d")
            acc_a = singles.tile([128, max(n_kept, 1)], fp32, tag="acc_a")
            n_units = 4096 // JW
            trash_d = singles.tile([128, JW], bf16, tag="trash_d")
            trash_a = singles.tile([128, JW], bf16, tag="trash_a")
            trash32 = singles.tile([128, R], fp32, tag="trash32")
            m_final = singles.tile([128, 2], fp32, tag="m_final")
            ones_sb = singles.tile([128, 1], fp32, tag="ones_sb")
            red_d = singles.tile([128, 1], fp32, tag="red_d")
            red_a = singles.tile([128, 1], fp32, tag="red_a")
            out_sb = singles.tile([2, 1], fp32, tag="out_sb")

            nc.vector.memset(ones_sb[:, :], 1.0)

            # ---- exact diagonal: sum_i ||f1_i - f2_i||^2 (fp32); runs
            # early on ACT, overlapped with the main loop ----
            nc.scalar.activation(
                trash32[:, :],
                s_diff[:, :],
                Act.Square,
                accum_out=acc_diag[:, 0:1],
            )

            # ---- main loop ----
            # One [128, 4096] PSUM tile = all 8 banks, used as 4 circular
            # 1024-wide units.  PE fills unit (st % 4) while earlier units
            # are screened; screens alternate DVE (even st) / ACT (odd st)
            # so both engines run concurrently on different units.
            order = [
                (ti, jh, keep[ti * NJH + jh])
                for ti in range(NCORES)
                for jh in range(NJH)
                if keep[ti * NJH + jh]
            ]
            # Greedy DVE/ACT assignment by measured per-op cost so the
            # mixed 512/1024-wide screens stay balanced across engines.
            def op_cost(fd, eng):
                if eng == "dve":
                    return 216.0 + fd / 0.96 + 263.0
                return 216.0 + fd / 1.2 + 583.0

            busy = {"dve": 0.0, "act": 0.0}
            engine_of = []
            for _, _, mode in order:
                fd = 512 * bin(mode).count("1")
                pick = min(("dve", "act"),
                           key=lambda e: busy[e] + op_cost(fd, e))
                engine_of.append(pick)
                busy[pick] += op_cost(fd, pick)

            with tc.tile_pool(name="psum_main", bufs=1, space="PSUM") as pp:
                big = pp.tile([128, 4096], fp32, tag="big")
                i_d = 0
                i_a = 0
                for st, (ti, jh, mode) in enumerate(order):
                    isl = slice(ti * 128, (ti + 1) * 128)
                    half = (st % n_units) * JW
                    # main matmuls for the kept 512-halves, packed from the
                    # unit start: mode 1 = lo half, 2 = hi half, 3 = both
                    halves = {1: (0,), 2: (1,), 3: (0, 1)}[mode]
                    for k, hv in enumerate(halves):
                        nc.tensor.matmul(
                            big[:, half + k * 512 : half + (k + 1) * 512],
                            lhsT=s_f1t[:, isl],
                            rhs=f2t2_slice(jh, hv),
                            start=True,
                            stop=True,
                        )
                    fd = 512 * len(halves)
                    # screen: relu(psum + bias_col) accumulated; zero iff
                    # no hinge term among the screened columns.
                    bias_col = s_s1c[:, st : st + 1]
                    if engine_of[st] == "dve":
                        nc.vector.tensor_scalar(
                            trash_d[:, 0:fd],
                            big[:, half : half + fd],
                            bias_col,
                            0.0,
                            Alu.add,
                            Alu.max,
                            accum_out=acc_d[:, i_d : i_d + 1],
                        )
                        i_d += 1
                    else:
                        nc.scalar.activation(
                            trash_a[:, 0:fd],
                            big[:, half : half + fd],
                            Act.Relu,
                            bias=bias_col,
                            scale=1.0,
                            accum_out=acc_a[:, i_a : i_a + 1],
                        )
                        i_a += 1

            # ---- final reduction ----
            nc.vector.tensor_reduce(
                red_d[:, :], acc_d[:, :], axis=mybir.AxisListType.X, op=Alu.add
            )
            nc.vector.tensor_reduce(
                red_a[:, :], acc_a[:, :], axis=mybir.AxisListType.X, op=Alu.add
            )
            nc.vector.tensor_copy(m_final[:, 0:1], acc_diag[:, 0:1])
            nc.vector.tensor_add(m_final[:, 1:2], red_d[:, :], red_a[:, :])

            with tc.tile_pool(name="psum_fin", bufs=1, space="PSUM") as pf_pool:
                pf = pf_pool.tile([2, 1], fp32, tag="pf")
                nc.tensor.matmul(
                    pf[:, :], lhsT=m_final[:, :], rhs=ones_sb[:, :],
                    start=True, stop=True,
                )
                nc.vector.tensor_copy(out_sb[:, :], pf[:, :])

            nc.sync.dma_start(d_out[:, :], out_sb[:, :])

    nc.compile()
    return nc


def _get_nc(keep):
    keep = tuple(bool(k) for k in keep)
    if keep not in _BASS_CACHE:
        _BASS_CACHE[keep] = _build_bass(keep)
    return _BASS_CACHE[keep]


def _full_numpy_fallback(f1, f2):
    """Exact reference computation (only used if the screen certificate
    fails, i.e. some pair has d_ij close to or inside the margin)."""
    f1 = f1.astype(np.float32)
    f2 = f2.astype(np.float32)
    n = f1.shape[0]
    sq1 = np.sum(f1 * f1, axis=1)
    sq2 = np.sum(f2 * f2, axis=1)
    total = np.float64(0.0)
    chunk = 512
    for s in range(0, n, chunk):
        e = min(s + chunk, n)
        d2 = sq1[s:e, None] + sq2[None, :] - 2.0 * (f1[s:e] @ f2.T)
        d = np.sqrt(np.maximum(d2, 0.0))
        c = np.maximum(1.0 - d, 0.0)
        for r in range(s, e):
            c[r - s, r] = 0.0
        total += np.float64(np.sum(c * c))
    total += np.float64(np.sum((f1 - f2) ** 2))
    return np.float32(total / (2.0 * n))


def kernel(feature1, feature2):
    global LAST_RESULT
    from concourse.bass_utils import run_bass_kernel_spmd

    f1 = np.ascontiguousarray(np.asarray(feature1, dtype=np.float32))
    f2 = np.ascontiguousarray(np.asarray(feature2, dtype=np.float32))
    assert f1.shape == (N, D) and f2.shape == (N, D)

    bf16 = ml_dtypes.bfloat16
    sq1 = np.sum(f1.astype(np.float64) * f1, axis=1)
    sq2 = np.sum(f2.astype(np.float64) * f2, axis=1)

    # Sort feature2 rows by sq2 so the per-supertile min-sq2 bias is tight.
    perm = np.argsort(sq2, kind="stable")
    f2s = f2[perm]
    sq2s = sq2[perm]
    sq2min = sq2s.reshape(NJH, JW).min(axis=1)  # per j-group minimum
    sq2max = sq2s.reshape(NJH, JW).max(axis=1)

    f2t2 = np.ascontiguousarray((2.0 * f2s.T).astype(bf16))           # [D, N]

    # Shard feature1 by striping the globally-sq1-sorted rows (core c gets
    # sorted rows c::8) so every core's i-tile ti covers the same norm
    # quantile band and the block-skip pattern is core-invariant.
    perm1 = np.argsort(sq1, kind="stable")
    rowids = [perm1[c::NCORES] for c in range(NCORES)]

    # Cauchy-Schwarz block certificate: a span (ti, jh) needs no screening
    # if |norm(f1_i) - norm(f2_j)| >= 1 for all pairs, i.e. the norm
    # intervals are separated by >= 1 (then d2 >= (n1-n2)^2 >= 1 exactly).
    # per-512-group norm intervals (sq2s ascending -> min is first elem)
    g2min = sq2s.reshape(16, 512).min(axis=1)
    g2max = sq2s.reshape(16, 512).max(axis=1)
    keep = []
    for ti in range(R // 128):
        n1lo = np.sqrt(min(sq1[rowids[c][ti * 128]] for c in range(NCORES)))
        n1hi = np.sqrt(max(sq1[rowids[c][(ti + 1) * 128 - 1]]
                           for c in range(NCORES)))
        for jh in range(NJH):
            mode = 0
            for hv in (0, 1):
                g = jh * 2 + hv
                n2lo, n2hi = np.sqrt(g2min[g]), np.sqrt(g2max[g])
                certified = (n2lo - n1hi >= 1.0 + 1e-6) or (
                    n1lo - n2hi >= 1.0 + 1e-6
                )
                if not certified:
                    mode |= 1 << hv
            keep.append(mode)
    kept_idx = [k for k, m in enumerate(keep) if m]

    in_maps = []
    for c in range(NCORES):
        rid = rowids[c]
        f1c_rows = f1[rid]                                            # [R, D]
        sq1c = sq1[rid]
        s1c = np.empty((128, len(kept_idx)), np.float32)
        for col, k in enumerate(kept_idx):
            ti, jh = k // NJH, k % NJH
            first_half = 0 if (keep[k] & 1) else 1
            s1c[:, col] = (
                1.0
                - sq1c[ti * 128 : (ti + 1) * 128]
                - g2min[jh * 2 + first_half]
            )
        in_maps.append(
            {
                "f2t2": f2t2,
                "f1t": np.ascontiguousarray(f1c_rows.T.astype(bf16)),
                "s1c": np.ascontiguousarray(s1c),
                "diff": np.ascontiguousarray(
                    f1c_rows.reshape(128, R) - f2[rid].reshape(128, R)
                ),
            }
        )

    nc = _get_nc(keep)
    res = run_bass_kernel_spmd(
        nc,
        in_maps,
        core_ids=list(range(NCORES)),
        trace=TRACE,
        **TRACE_KWARGS,
    )
    LAST_RESULT = res

    diag_total = np.float64(0.0)
    screen_total = np.float64(0.0)
    for r in res.results:
        out = r["out"]
        diag_total += np.float64(out[0, 0])
        screen_total += np.float64(out[1, 0])

    if screen_total != 0.0:
        return _full_numpy_fallback(f1, f2)

    return np.float32(diag_total / (2.0 * N))
